# revision 1
# baseline (speedup 1.0000x reference)
"""Trainium2 Bass kernel for nn_MemTransformerLM (hourglass Transformer-XL).

Sharding: 8 cores = 4 batch rows x 2-way tensor parallel (heads / d_inner /
vocab halves). Both cores of a pair hold the full residual stream; per-layer
partial outputs (o-proj, ff2) are summed with AllReduce-2 over core pairs.

Activations flow transposed ("T-layout": model dim on partitions, tokens on
the free axis). Weights are host-pre-transposed to contraction-major bf16.
The Transformer-XL rel_shift runs on GPSIMD local_scatter (per-partition
staircase indices; negative index = causal drop). Softmax skips the max
subtraction (scores provably small); denominators fall out of the Exp
activation's accum_out during PSUM eviction.
"""
import os
import sys
sys.path.insert(0, '/opt/trn_rl_repo')

import numpy as np
import ml_dtypes

import concourse.bass as bass
import concourse.tile as tile
from concourse import bacc, mybir
from concourse.bass_utils import run_bass_kernel_spmd

F32 = mybir.dt.float32
F32R = mybir.dt.float32r
BF16 = mybir.dt.bfloat16
I16 = mybir.dt.int16
AF = mybir.ActivationFunctionType
ALU = mybir.AluOpType

T, B, D, H, DH, DI, V, L = 1024, 4, 512, 8, 64, 2048, 256, 8
STAGES = (2, 4, 2)
S = 256
SP = 384          # padded short length (3 tiles; real rows 0..256)
KD = D // 128     # 4 d-tiles
HO = H // 2       # 4 own heads per core
QC = HO * DH      # 256 own q/k/v columns
DIO = DI // 2     # 1024 own ff-inner dims
NKI = DIO // 128  # 8 ff-inner k-tiles
NEG = -1.0e30
SCALE = 0.125

N_CORES = 8
REPS = int(os.environ.get('KERNEL_REPS', '1'))


def ts(i, n=128):
    return slice(i * n, (i + 1) * n)


def chunk_list(NT):
    return [(c * 512, min(512, NT - c * 512)) for c in range((NT + 511) // 512)]


def _ln(nc, p, psum, xpre, NT, g, b):
    """LayerNorm over the partition (d) axis in T-layout.
    Stats via ones-matmuls; a/b broadcast rows shared across the 4 d-tiles."""
    chunks = chunk_list(NT)
    ones_b = _ln.ones_b
    ones_row = _ln.ones_row
    arow = p.tile([1, NT], F32, tag="arow")
    brow = p.tile([1, NT], F32, tag="brow")
    for c0, cw in chunks:
        ps1 = psum.tile([1, 512], F32, tag="st1")[:, :cw]
        ps2 = psum.tile([1, 512], F32, tag="st2")[:, :cw]
        for m in range(KD):
            xbf = p.tile([128, 512], BF16, tag="xbf")[:, :cw]
            nc.vector.tensor_copy(xbf, xpre[m][:, c0:c0 + cw])
            nc.tensor.matmul(ps1, ones_b, xbf, start=(m == 0), stop=(m == KD - 1))
            sqm = xbf
            nc.scalar.activation(sqm, xpre[m][:, c0:c0 + cw], AF.Square)
            nc.tensor.matmul(ps2, ones_b, sqm, start=(m == 0), stop=(m == KD - 1))
        mean = p.tile([1, 512], F32, tag="mean")[:, :cw]
        var = p.tile([1, 512], F32, tag="var")[:, :cw]
        msq = arow[:, c0:c0 + cw]
        nc.vector.tensor_scalar_mul(mean, ps1, 1.0 / D)
        nc.vector.tensor_scalar_mul(var, ps2, 1.0 / D)
        nc.vector.tensor_tensor(msq, mean, mean, ALU.mult)
        nc.vector.tensor_tensor(var, var, msq, ALU.subtract)
        nc.vector.tensor_scalar_add(var, var, 1.0e-5)
        nc.vector.reciprocal(var, var)
        nc.scalar.activation(arow[:, c0:c0 + cw], var, AF.Sqrt)
        nc.vector.tensor_tensor(brow[:, c0:c0 + cw], mean, arow[:, c0:c0 + cw], ALU.mult)
        nc.vector.tensor_scalar_mul(brow[:, c0:c0 + cw], brow[:, c0:c0 + cw], -1.0)
    XF, XB = [], []
    for m in range(KD):
        XF.append(p.tile([128, NT], F32, tag="XF", name=f"XF{m}"))
        XB.append(p.tile([128, NT], BF16, tag="XB", name=f"XB{m}"))
    for ci, (c0, cw) in enumerate(chunks):
        aps = psum.tile([128, 512], F32, tag="sc")[:, :cw]
        bps = psum.tile([128, 512], F32, tag="sc")[:, :cw]
        nc.tensor.matmul(aps, ones_row, arow[:, c0:c0 + cw], start=True, stop=True)
        nc.tensor.matmul(bps, ones_row, brow[:, c0:c0 + cw], start=True, stop=True)
        asb = p.tile([128, 512], F32, tag="lnab", name=f"asb{ci}")[:, :cw]
        bsb = p.tile([128, 512], F32, tag="lnab", name=f"bsb{ci}")[:, :cw]
        nc.scalar.copy(asb, aps)
        nc.scalar.copy(bsb, bps)
        for m in range(KD):
            t1 = p.tile([128, 512], F32, tag="lnt")[:, :cw]
            nc.vector.tensor_tensor(t1, xpre[m][:, c0:c0 + cw], asb, ALU.mult)
            nc.vector.tensor_tensor(t1, t1, bsb, ALU.add)
            nc.scalar.activation(XF[m][:, c0:c0 + cw], t1, AF.Identity,
                                 bias=b[:, m:m + 1], scale=g[:, m:m + 1])
    for m in range(KD):
        nc.vector.tensor_copy(XB[m][:], XF[m][:])
    return XF, XB


def _allreduce(nc, p, dram, fused, n, NT, tag):
    """fused: single [128, n, NT] SBUF bf16 tile. Returns it, allreduced."""
    bin_ = dram.tile([128, n, NT], BF16, tag=f"ari_{tag}")
    bout = dram.tile([128, n, NT], BF16, tag=f"aro_{tag}")
    nc.sync.dma_start(bin_[:], fused[:])
    nc.gpsimd.collective_compute(
        "AllReduce", ALU.add,
        replica_groups=[[0, 1], [2, 3], [4, 5], [6, 7]],
        ins=[bin_.opt()], outs=[bout.opt()])
    nc.sync.dma_start(fused[:], bout[:])
    return fused


def _layer(nc, pools, lw, XF, XB, NTT, consts):
    p, psum, dram = pools
    NT = NTT * 128
    chunks = chunk_list(NT)
    idbf, idf, sinT, rwb, rrb = (consts[k] for k in ('idbf', 'idf', 'sinT', 'rwb', 'rrb'))

    # --- uT: host-precomputed rk projection (position-only), DMA per layer ---
    uT = []
    for m in range(2):
        u = p.tile([128, NT], BF16, tag="uT")
        nc.sync.dma_start(u[:], lw['uTd'].ap()[lw['li'], ts(m), :NT])
        uT.append(u)

    # --- qkv projections (q/k in T-layout; v in N-layout) ---
    qac, qbd, kb = [], [], []
    for m in range(2):
        qa = p.tile([128, NT], BF16, tag="qac")
        qb = p.tile([128, NT], BF16, tag="qbd")
        kk = p.tile([128, NT], BF16, tag="kb")
        for c0, cw in chunks:
            ps = psum.tile([128, 512], F32, tag="sc")[:, :cw]
            for kd in range(KD):
                nc.tensor.matmul(ps, lw['wqkvT'][kd][:, ts(m)], XB[kd][:, c0:c0 + cw],
                                 start=(kd == 0), stop=(kd == KD - 1))
            nc.scalar.activation(qa[:, c0:c0 + cw], ps, AF.Identity, bias=rwb[:, m:m + 1])
            nc.scalar.activation(qb[:, c0:c0 + cw], ps, AF.Identity, bias=rrb[:, m:m + 1])
            ps2 = psum.tile([128, 512], F32, tag="sc")[:, :cw]
            for kd in range(KD):
                nc.tensor.matmul(ps2, lw['wqkvT'][kd][:, ts(m + 2)], XB[kd][:, c0:c0 + cw],
                                 start=(kd == 0), stop=(kd == KD - 1))
            nc.scalar.copy(kk[:, c0:c0 + cw], ps2)
        qac.append(qa)
        qbd.append(qb)
        kb.append(kk)

    vb = []
    for tt in range(NTT):
        v = p.tile([128, QC], BF16, tag="vb", name=f"vb{tt}")
        ps = psum.tile([128, 512], F32, tag="sc")[:, :QC]
        for kd in range(KD):
            nc.tensor.matmul(ps, XB[kd][:, ts(tt)], lw['wqkvT'][kd][:, 512:768],
                             start=(kd == 0), stop=(kd == KD - 1))
        nc.vector.tensor_copy(v[:], ps)
        vb.append(v)

    # --- attention: q-groups of 4 tiles, head-stacked scatter, PE transposes,
    # wide-N PV ---
    voT = [p.tile([128, NT], BF16, tag="voT", name=f"voT{m}") for m in range(2)]
    NG = (NTT + 3) // 4
    for g in range(NG):
        q_lo = 4 * g
        q_hi = min(4 * g + 4, NTT)
        gw = (q_hi - q_lo) * 128
        extw = [p.tile([128, HO, 512], BF16, tag=f"ext{jt}", name=f"extw{jt}")
                for jt in range(q_hi)]
        for qi in range(q_lo, q_hi):
            W = (qi + 1) * 128
            i0 = qi * 128
            qoff = (qi - q_lo) * 128
            idx = p.tile([128, 1024], I16, tag="idx")
            nc.gpsimd.iota(idx[:, :W], pattern=[[-1, W]], base=i0,
                           channel_multiplier=1)
            wch = [(c0, min(cw, W - c0)) for c0, cw in chunks if c0 < W]
            den_mat = p.tile([128, HO], F32, tag="den")
            bdd = p.tile([128, HO, 1024], BF16, tag="bdd")
            for c0, cw in wch:
                bdps = psum.tile([128, HO, 512], F32, tag="pv")
                for hh in range(HO):
                    mi, po = hh // 2, (hh % 2) * 64
                    nc.tensor.matmul(bdps[:, hh, :cw],
                                     qbd[mi][po:po + 64, i0:i0 + 128],
                                     uT[mi][po:po + 64, c0:c0 + cw],
                                     start=True, stop=True)
                nc.scalar.copy(bdd[:, :, c0:c0 + cw], bdps[:, :, :cw])
            bds = p.tile([128, HO, 1024], BF16, tag="bds")
            for hh in range(HO):
                nc.gpsimd.local_scatter(bds[:, hh, :W], bdd[:, hh, :W], idx[:, :W],
                                        channels=128, num_elems=W, num_idxs=W)
            nc.gpsimd.affine_select(bds[:, :, i0:W], bds[:, :, i0:W],
                                    pattern=[[0, HO], [-1, 128]],
                                    compare_op=ALU.is_ge, fill=NEG,
                                    base=0, channel_multiplier=1)
            ex_all = p.tile([128, HO, 1024], BF16, tag="exall")
            den2 = p.tile([128, HO, 2], F32, tag="den2")
            for hh in range(HO):
                mi, po = hh // 2, (hh % 2) * 64
                for ci, (c0, cw) in enumerate(wch):
                    ps = psum.tile([128, 512], F32, tag="sc")[:, :cw]
                    nc.tensor.matmul(ps, idbf, bds[:, hh, c0:c0 + cw],
                                     start=True, stop=False)
                    nc.tensor.matmul(ps, qac[mi][po:po + 64, i0:i0 + 128],
                                     kb[mi][po:po + 64, c0:c0 + cw],
                                     start=False, stop=True)
                    nc.scalar.activation(ex_all[:, hh, c0:c0 + cw], ps, AF.Exp,
                                         scale=SCALE, accum_out=den2[:, hh, ci:ci + 1])
            if len(wch) == 1:
                nc.vector.reciprocal(den_mat[:], den2[:, :, 0])
            else:
                nc.vector.tensor_tensor(den_mat[:], den2[:, :, 0], den2[:, :, 1],
                                        ALU.add)
                nc.vector.reciprocal(den_mat[:], den_mat[:])
            for hh in range(HO):
                nc.vector.tensor_scalar_mul(ex_all[:, hh, :W], ex_all[:, hh, :W],
                                            den_mat[:, hh:hh + 1])
            for jt in range(qi + 1):
                pst = psum.tile([128, HO, 128], BF16, tag="sc")
                for hh in range(HO):
                    nc.tensor.transpose(pst[:, hh], ex_all[:, hh, ts(jt)], idbf)
                nc.scalar.copy(extw[jt][:, :, qoff:qoff + 128], pst[:])
        # PV over the whole group, per head
        pvps = psum.tile([64, HO, 512], F32, tag="pv")
        for hh in range(HO):
            for jt in range(q_hi):
                coloff = max(0, jt - q_lo) * 128
                nc.tensor.matmul(pvps[:, hh, coloff:gw],
                                 vb[jt][:, hh * 64:hh * 64 + 64],
                                 extw[jt][:, hh, coloff:gw],
                                 start=(jt == 0), stop=(jt == q_hi - 1),
                                 skip_group_check=True)
            mi, po = hh // 2, (hh % 2) * 64
            nc.vector.tensor_copy(voT[mi][po:po + 64, q_lo * 128:q_lo * 128 + gw],
                                  pvps[:, hh, :gw])

    # --- o-proj partial + AllReduce + residual + LN1 ---
    obuf = p.tile([128, KD, NT], BF16, tag="arb")
    for c0, cw in chunks:
        ops_ = psum.tile([128, KD, 512], F32, tag="pv")
        for m in range(KD):
            for kt in range(2):
                nc.tensor.matmul(ops_[:, m, :cw], lw['woT'][kt][:, ts(m)],
                                 voT[kt][:, c0:c0 + cw],
                                 start=(kt == 0), stop=(kt == 1))
        nc.vector.tensor_copy(obuf[:, :, c0:c0 + cw], ops_[:, :, :cw])
    oar = _allreduce(nc, p, dram, obuf, KD, NT, tag=f"o{lw['li']}")
    xpre = []
    for m in range(KD):
        xp = p.tile([128, NT], F32, tag="big", name=f"xp{m}")
        nc.vector.tensor_tensor(xp[:], oar[:, m], XF[m][:], ALU.add)
        xpre.append(xp)
    XF, XB = _ln(nc, p, psum, xpre, NT, lw['g1'], lw['bb1'])

    # --- FFN ---
    hb = []
    for m in range(NKI):
        hbt = p.tile([128, NT], BF16, tag="hb", name=f"hb{m}")
        for c0, cw in chunks:
            ps = psum.tile([128, 512], F32, tag="sc")[:, :cw]
            for kd in range(KD):
                nc.tensor.matmul(ps, lw['w1T'][kd][:, ts(m)], XB[kd][:, c0:c0 + cw],
                                 start=(kd == 0), stop=(kd == KD - 1))
            nc.scalar.activation(hbt[:, c0:c0 + cw], ps, AF.Relu,
                                 bias=lw['fb1'][:, m:m + 1])
        hb.append(hbt)
    fbuf = p.tile([128, KD, NT], BF16, tag="arb")
    for c0, cw in chunks:
        fps = psum.tile([128, KD, 512], F32, tag="pv")
        for m in range(KD):
            for kt in range(NKI):
                nc.tensor.matmul(fps[:, m, :cw], lw['w2T'][kt][:, ts(m)],
                                 hb[kt][:, c0:c0 + cw],
                                 start=(kt == 0), stop=(kt == NKI - 1))
        nc.vector.tensor_copy(fbuf[:, :, c0:c0 + cw], fps[:, :, :cw])
    far = _allreduce(nc, p, dram, fbuf, KD, NT, tag=f"f{lw['li']}")
    xpre2 = []
    for m in range(KD):
        xp = p.tile([128, NT], F32, tag="big", name=f"xq{m}")
        t1 = p.tile([128, NT], F32, tag="lnt")
        nc.scalar.activation(t1[:], far[:, m], AF.Identity, bias=lw['fb2'][:, m:m + 1])
        nc.vector.tensor_tensor(xp[:], t1[:], XF[m][:], ALU.add)
        xpre2.append(xp)
    return _ln(nc, p, psum, xpre2, NT, lw['g2'], lw['bb2'])


def build_program():
    nc = bacc.Bacc("TRN2", target_bir_lowering=False, debug=False, num_devices=N_CORES)
    d = {}

    def di(name, shape, dt):
        d[name] = nc.dram_tensor(name, shape, dt, kind="ExternalInput")

    di("wqkvT", [L, D, 3 * QC], BF16)
    di("wrkT", [L, D, QC], BF16)
    di("uTall", [L, QC, T], BF16)
    di("woT", [L, QC, D], BF16)
    di("w1T", [L, D, DIO], BF16)
    di("w2T", [L, DIO, D], BF16)
    di("fb1", [L, DIO], F32)
    di("fb2", [L, D], F32)
    di("g1", [L, D], F32)
    di("bb1", [L, D], F32)
    di("g2", [L, D], F32)
    di("bb2", [L, D], F32)
    di("rwb", [QC], F32)
    di("rrb", [QC], F32)
    di("wemb", [V, D], BF16)
    di("onehotT", [V, T], BF16)
    di("sinTd", [D, T], BF16)
    di("idbf", [128, 128], BF16)
    di("idf", [128, 128], F32)
    di("wpool", [T, SP], BF16)
    di("nullv", [D], F32)
    di("gd", [D], F32)
    di("bdn", [D], F32)
    di("uup", [SP, T], BF16)
    di("finT", [D, V // 2], BF16)
    di("fbn", [V // 2], F32)
    logits = nc.dram_tensor("logits", [T, V // 2], F32, kind="ExternalOutput")

    with tile.TileContext(nc) as tc:
        import itertools
        _ctr = itertools.count()

        class NP:
            def __init__(self, pool):
                self.pool = pool

            def tile(self, shape, dt, tag=None, name=None):
                if name is None:
                    name = f"{tag}_{next(_ctr)}"
                return self.pool.tile(shape, dt, tag=tag, name=name)

        with tc.tile_pool(name="p", bufs=2) as p_r, \
             tc.tile_pool(name="pbig", bufs=4) as pbig_r, \
             tc.tile_pool(name="px", bufs=4) as px_r, \
             tc.tile_pool(name="pr", bufs=4) as pr_r, \
             tc.tile_pool(name="ph", bufs=8) as ph_r, \
             tc.tile_pool(name="pw", bufs=1) as pw_r, \
             tc.tile_pool(name="pw1", bufs=1) as pw1_r, \
             tc.tile_pool(name="pc", bufs=1) as pc_r, \
             tc.tile_pool(name="pb1", bufs=1) as pb1_r, \
             tc.tile_pool(name="psum", bufs=2, space="PSUM") as psum_r, \
             tc.tile_pool(name="psum1", bufs=1, space="PSUM") as psum1_r, \
             tc.tile_pool(name="dram", bufs=1, space="DRAM") as dram_r:
            p, pbig, px, pr, ph, pw, pw1, pc, pb1 = (NP(x) for x in
                                            (p_r, pbig_r, px_r, pr_r, ph_r, pw_r,
                                             pw1_r, pc_r, pb1_r))
            psum_, psum1_, dram = NP(psum_r), NP(psum1_r), NP(dram_r)

            # pool router: route tags to pools with the right bufs counts
            class P:
                def tile(self, shape, dt, tag=None, name=None):
                    if tag in ("XF", "XB"):
                        return px.tile(shape, dt, tag=tag, name=name)
                    if tag in ("big", "residF"):
                        return pbig.tile(shape, dt, tag=tag, name=name)
                    if tag in ("hb", "vb"):
                        return ph.tile(shape, dt, tag=tag, name=name)
                    if tag in ("arb", "arout") or (tag is not None
                                                   and tag.startswith("ext")):
                        return pw1.tile(shape, dt, tag=tag, name=name)
                    if tag in ("bdd", "bds", "exall", "xbf", "sq", "louts"):
                        return pb1.tile(shape, dt, tag=tag, name=name)
                    if tag in ("idx", "arow", "brow", "mean", "var", "rv", "msq", "dro"):
                        return pc.tile(shape, dt, tag=tag, name=name)
                    if tag is not None and tag.startswith("c_"):
                        return pc.tile(shape, dt, tag=tag, name=name)
                    return p.tile(shape, dt, tag=tag, name=name)
            pp = P()

            class PS:
                def tile(self, shape, dt, tag=None, name=None):
                    if tag in ("st1", "st2", "pv"):
                        return psum1_.tile(shape, dt, tag=tag, name=name)
                    return psum_.tile(shape, dt, tag=tag, name=name)
            pps = PS()
            pools = (pp, pps, dram)

            consts = {}
            idbf = pc.tile([128, 128], BF16, tag="c_idbf")
            nc.sync.dma_start(idbf[:], d["idbf"].ap())
            idf = None
            ones_b = pc.tile([128, 1], BF16, tag="c_ones")
            nc.gpsimd.memset(ones_b[:], 1.0)
            ones_row = pc.tile([1, 128], F32, tag="c_onesr")
            nc.gpsimd.memset(ones_row[:], 1.0)
            _ln.ones_b = ones_b
            _ln.ones_row = ones_row
            sinT = None
            rwb = pc.tile([128, 2], F32, tag="c_rwb")
            nc.sync.dma_start(rwb[:], d["rwb"].ap().rearrange("(a q) -> q a", q=128))
            rrb = pc.tile([128, 2], F32, tag="c_rrb")
            nc.sync.dma_start(rrb[:], d["rrb"].ap().rearrange("(a q) -> q a", q=128))
            consts.update(idbf=idbf, idf=idf, sinT=sinT, rwb=rwb, rrb=rrb)

            def load_layer(li, rep):
                lw = {'li': li, 'rep': rep, 'uTd': d["uTall"]}
                lw['wqkvT'] = [pw.tile([128, 3 * QC], BF16, tag=f"w_qkv{k}")
                               for k in range(KD)]
                lw['woT'] = [pw.tile([128, D], BF16, tag=f"w_o{k}")
                             for k in range(2)]
                lw['w1T'] = [pw1.tile([128, DIO], BF16, tag=f"w_1{k}")
                             for k in range(KD)]
                lw['w2T'] = [pw1.tile([128, D], BF16, tag=f"w_2{k}")
                             for k in range(NKI)]
                for k in range(KD):
                    nc.sync.dma_start(lw['wqkvT'][k][:], d["wqkvT"].ap()[li, ts(k), :])
                    nc.sync.dma_start(lw['w1T'][k][:], d["w1T"].ap()[li, ts(k), :])
                for k in range(2):
                    nc.sync.dma_start(lw['woT'][k][:], d["woT"].ap()[li, ts(k), :])
                for k in range(NKI):
                    nc.sync.dma_start(lw['w2T'][k][:], d["w2T"].ap()[li, ts(k), :])
                for nm in ("fb1", "fb2", "g1", "bb1", "g2", "bb2"):
                    cols = NKI if nm == "fb1" else KD
                    tl = pw.tile([128, cols], F32, tag=f"w_{nm}")
                    nc.sync.dma_start(tl[:], d[nm].ap()[li].rearrange("(a q) -> q a", q=128))
                    lw[nm] = tl
                return lw

            from contextlib import ExitStack as _ES

            def _rep_body(rep):
                # --- embedding (one-hot matmul) ---
                XF, XB = [], []
                for m in range(KD):
                    xf = pp.tile([128, T], F32, tag="XF", name=f"XF{m}")
                    xb = pp.tile([128, T], BF16, tag="XB", name=f"XB{m}")
                    wemb = pp.tile([128, 2, 128], BF16, tag="vb")
                    nc.sync.dma_start(
                        wemb[:],
                        d["wemb"].ap().rearrange("(a q) e -> q a e", q=128)[:, :, ts(m)])
                    for c in range(2):
                        ps = pps.tile([128, 512], F32, tag="sc")
                        oh = pp.tile([128, 2, 512], BF16, tag="hb")
                        nc.sync.dma_start(
                            oh[:],
                            d["onehotT"].ap().rearrange("(a q) t -> q a t", q=128)
                            [:, :, c * 512:(c + 1) * 512])
                        for vk in range(2):
                            nc.tensor.matmul(ps, wemb[:, vk], oh[:, vk],
                                             start=(vk == 0), stop=(vk == 1))
                        nc.vector.tensor_copy(xf[:, c * 512:(c + 1) * 512], ps)
                        nc.scalar.copy(xb[:, c * 512:(c + 1) * 512], ps)
                    XF.append(xf)
                    XB.append(xb)

                for li in range(STAGES[0]):
                    XF, XB = _layer(nc, pools, load_layer(li, rep), XF, XB, T // 128, consts)

                residF = []
                for m in range(KD):
                    r = pbig.tile([128, T], BF16, tag="residF", name=f"res{m}")
                    nc.vector.tensor_copy(r[:], XF[m][:])
                    residF.append(r)

                # --- downsample ---
                XN = []
                for tt in range(T // 128):
                    xn = pp.tile([128, D], BF16, tag="hb", name=f"xn{tt}")
                    for m in range(KD):
                        pt = pps.tile([128, 128], BF16, tag="sc")
                        nc.tensor.transpose(pt[:], XB[m][:, ts(tt)], idbf)
                        nc.vector.tensor_copy(xn[:, ts(m)], pt[:])
                    XN.append(xn)
                wpool = [pp.tile([128, SP], BF16, tag="vb", name=f"pl{tt}")
                         for tt in range(T // 128)]
                for tt in range(T // 128):
                    nc.sync.dma_start(wpool[tt][:], d["wpool"].ap()[ts(tt), :])
                nullv = pc.tile([128, KD], F32, tag="c_null")
                nc.sync.dma_start(nullv[:], d["nullv"].ap().rearrange("(a q) -> q a", q=128))
                spre = []
                for m in range(KD):
                    sp_ = pbig.tile([128, SP], F32, tag="big", name=f"sp{m}")
                    ps = pps.tile([128, 512], F32, tag="sc")[:, :SP]
                    for tt in range(T // 128):
                        nc.tensor.matmul(ps, XN[tt][:, ts(m)], wpool[tt][:],
                                         start=(tt == 0), stop=(tt == T // 128 - 1))
                    nc.vector.tensor_copy(sp_[:], ps)
                    nc.vector.tensor_copy(sp_[:, 0:1], nullv[:, m:m + 1])
                    spre.append(sp_)
                gdt = pc.tile([128, KD], F32, tag="c_gd")
                nc.sync.dma_start(gdt[:], d["gd"].ap().rearrange("(a q) -> q a", q=128))
                bdt = pc.tile([128, KD], F32, tag="c_bd")
                nc.sync.dma_start(bdt[:], d["bdn"].ap().rearrange("(a q) -> q a", q=128))
                SXF, SXB = _ln(nc, pp, pps, spre, SP, gdt, bdt)

                for li in range(STAGES[0], STAGES[0] + STAGES[1]):
                    SXF, SXB = _layer(nc, pools, load_layer(li, rep), SXF, SXB, SP // 128, consts)

                # --- upsample + residual ---
                SN = []
                for st in range(SP // 128):
                    sn = pp.tile([128, D], BF16, tag="hb", name=f"sn{st}")
                    for m in range(KD):
                        pt = pps.tile([128, 128], BF16, tag="sc")
                        nc.tensor.transpose(pt[:], SXB[m][:, ts(st)], idbf)
                        nc.vector.tensor_copy(sn[:, ts(m)], pt[:])
                    SN.append(sn)
                uup = [pp.tile([128, T], BF16, tag="hb", name=f"uu{st}")
                       for st in range(SP // 128)]
                for st in range(SP // 128):
                    nc.sync.dma_start(uup[st][:], d["uup"].ap()[ts(st), :])
                XF2, XB2 = [], []
                for m in range(KD):
                    xf = pp.tile([128, T], F32, tag="XF", name=f"XF{m}")
                    xb = pp.tile([128, T], BF16, tag="XB", name=f"XB{m}")
                    for c in range(2):
                        ps = pps.tile([128, 512], F32, tag="sc")
                        for st in range(SP // 128):
                            nc.tensor.matmul(ps, SN[st][:, ts(m)],
                                             uup[st][:, c * 512:(c + 1) * 512],
                                             start=(st == 0), stop=(st == SP // 128 - 1))
                        nc.vector.tensor_tensor(xf[:, c * 512:(c + 1) * 512], ps,
                                                residF[m][:, c * 512:(c + 1) * 512], ALU.add)
                        nc.scalar.copy(xb[:, c * 512:(c + 1) * 512],
                                       xf[:, c * 512:(c + 1) * 512])
                    XF2.append(xf)
                    XB2.append(xb)
                XF, XB = XF2, XB2

                for li in range(STAGES[0] + STAGES[1], L):
                    XF, XB = _layer(nc, pools, load_layer(li, rep), XF, XB, T // 128, consts)

                # --- final vocab projection (own half) ---
                finT = [pp.tile([128, V // 2], BF16, tag="vb", name=f"fin{k}")
                        for k in range(KD)]
                for k in range(KD):
                    nc.sync.dma_start(finT[k][:], d["finT"].ap()[ts(k), :])
                fbn = pc.tile([1, V // 2], F32, tag="c_fbn")
                nc.sync.dma_start(fbn[:], d["fbn"].ap()[None, :])
                for tt in range(T // 128):
                    ps = pps.tile([128, 512], F32, tag="sc")[:, :V // 2]
                    for kd in range(KD):
                        nc.tensor.matmul(ps, XB[kd][:, ts(tt)], finT[kd][:],
                                         start=(kd == 0), stop=(kd == KD - 1))
                    nc.tensor.matmul(ps, ones_row, fbn[:], start=False, stop=True)
                    lo = p.tile([128, V // 2], F32, tag="louts")
                    nc.vector.tensor_copy(lo[:], ps)
                    nc.sync.dma_start(logits.ap()[ts(tt), :], lo[:])

            for rep in range(REPS):
                _rep_body(rep)

    nc.compile()
    return nc


def host_inputs(inputs):
    bf = lambda x: np.ascontiguousarray(np.asarray(x, dtype=np.float32)).astype(ml_dtypes.bfloat16)
    f32 = lambda x: np.ascontiguousarray(np.asarray(x), dtype=np.float32)
    qkv_w = f32(inputs['qkv_w'])
    rk_w = f32(inputs['rk_w'])
    o_w = f32(inputs['o_w'])
    ff_w1 = f32(inputs['ff_w1'])
    ff_w2 = f32(inputs['ff_w2'])
    data = np.asarray(inputs['data'])
    bnd = np.asarray(inputs['boundaries_gt'])

    inv = 1.0 / (10000.0 ** (np.arange(0, D, 2, dtype=np.float32) / D))
    ang = np.arange(T, dtype=np.float32)[:, None] * inv[None, :]
    sin_tab = np.concatenate([np.sin(ang), np.cos(ang)], -1).astype(np.float32)
    eye = np.eye(128, dtype=np.float32)

    in_maps = []
    for c in range(N_CORES):
        b, h = c // 2, c % 2
        heads = list(range(h * HO, h * HO + HO))
        qr = np.concatenate([np.arange(g * DH, (g + 1) * DH) for g in heads])
        di_own = np.arange(h * DIO, (h + 1) * DIO)
        v_own = np.arange(h * (V // 2), (h + 1) * (V // 2))

        im = {}
        im['wqkvT'] = bf(np.stack([qkv_w[l][np.concatenate([qr, 512 + qr, 1024 + qr])].T
                                   for l in range(L)]))
        im['wrkT'] = bf(np.stack([rk_w[l][qr].T for l in range(L)]))
        im['uTall'] = bf(np.stack([rk_w[l][qr] @ sin_tab.T for l in range(L)]))
        im['woT'] = bf(np.stack([o_w[l][:, qr].T for l in range(L)]))
        im['w1T'] = bf(np.stack([ff_w1[l][di_own].T for l in range(L)]))
        im['w2T'] = bf(np.stack([ff_w2[l][:, di_own].T for l in range(L)]))
        im['fb1'] = f32(np.asarray(inputs['ff_b1'])[:, di_own])
        im['fb2'] = f32(inputs['ff_b2'])
        im['g1'] = f32(inputs['ln1_g'])
        im['bb1'] = f32(inputs['ln1_b'])
        im['g2'] = f32(inputs['ln2_g'])
        im['bb2'] = f32(inputs['ln2_b'])
        im['rwb'] = f32(np.asarray(inputs['r_w_bias'])[heads].reshape(-1))
        im['rrb'] = f32(np.asarray(inputs['r_r_bias'])[heads].reshape(-1))
        im['wemb'] = bf(inputs['word_emb'])
        oh = np.zeros((V, T), np.float32)
        oh[data[:, b], np.arange(T)] = 1.0
        im['onehotT'] = bf(oh)
        im['sinTd'] = bf(sin_tab.T)
        im['idbf'] = bf(eye)
        im['idf'] = f32(eye)
        hard = bnd[:, b].astype(np.float32)
        seg = np.cumsum(hard) - hard
        ind = (seg[:, None] == np.arange(S)).astype(np.float32)
        wmat = ind / (ind.sum(0, keepdims=True) + 1e-9)
        wp_ = np.zeros((T, SP), np.float32)
        wp_[:, 1:1 + S] = wmat
        im['wpool'] = bf(wp_)
        im['nullv'] = f32(np.asarray(inputs['null_group']).reshape(-1))
        im['gd'] = f32(inputs['down_ln_g'])
        im['bdn'] = f32(inputs['down_ln_b'])
        segU = np.clip(np.cumsum(hard).astype(np.int64), 0, S)
        uu = np.zeros((SP, T), np.float32)
        uu[segU, np.arange(T)] = 1.0
        im['uup'] = bf(uu)
        im['finT'] = bf(f32(inputs['final_w'])[v_own].T)
        im['fbn'] = f32(np.asarray(inputs['final_b'])[v_own])
        in_maps.append(im)
    return in_maps


_NC_CACHE = {}


def get_program():
    if 'nc' not in _NC_CACHE:
        _NC_CACHE['nc'] = build_program()
    return _NC_CACHE['nc']


def kernel(**inputs) -> np.ndarray:
    nc = get_program()
    in_maps = host_inputs(inputs)
    res = run_bass_kernel_spmd(nc, in_maps, core_ids=list(range(N_CORES)), trace=False)
    out = np.zeros((T, B, V), np.float32)
    for c in range(N_CORES):
        b, h = c // 2, c % 2
        out[:, b, h * (V // 2):(h + 1) * (V // 2)] = res.results[c]['logits']
    return out

